# revision 12
# baseline (speedup 1.0000x reference)
"""Block-sparse position-wise FFN on Trainium2 (Bass/Tile), 8-core data-parallel.

Strategy:
  - Shard tokens (B*S = 36928) evenly across 8 cores: 4616 tokens/core.
    The FFN is pointwise over tokens and both (masked) weight matrices fit
    in SBUF, so data-parallel needs no collectives.
  - Host prep (free): apply the 8x8 block masks to W1/W2, cast everything to
    bf16, and pre-transpose x to [DIM, T] so the device does zero transposes.
    The fc2 output is produced in [DIM, T] layout and un-transposed on host.
  - Per core, fused loop over token chunks (<=512 tokens, one PSUM bank):
      h  = gelu(W1m^T-tiles @ xT + b1)      (bf16 matmuls, ACT gelu+bias)
      outT = W2m^T-tiles @ h + b2           (bf16 matmuls, DVE bias add)
    Both layers are weight-stationary; h stays in [ff, token] layout
    throughout, so no on-device transposition is ever needed.
"""

import sys
import types

import numpy as np
import ml_dtypes

# concourse's axon trace path imports antenv.axon_hooks, which this image
# lacks; install a no-op shim so an env-requested trace degrades gracefully
# instead of raising ImportError.
try:
    import antenv.axon_hooks  # noqa: F401
except ImportError:
    import antenv

    _hooks = types.ModuleType("antenv.axon_hooks")
    _hooks._hook = None
    _hooks.set_axon_ntff_profile_hook = (
        lambda h: setattr(_hooks, "_hook", h))
    _hooks.get_axon_ntff_profile_hook = lambda: _hooks._hook
    sys.modules["antenv.axon_hooks"] = _hooks
    antenv.axon_hooks = _hooks

import concourse.bass as bass
import concourse.bacc as bacc
import concourse.mybir as mybir
from concourse import tile
from concourse.bass_utils import run_bass_kernel_spmd

B, S, DIM, FF, BLK = 64, 577, 768, 3072, 8
NCORES = 8
TOK = B * S                # 36928
T = TOK // NCORES          # 4616 tokens per core
P = 128
KD = DIM // P              # 6 d-tiles (fc1 contraction / fc2 output)
KF = FF // P               # 24 f-tiles (fc1 output / fc2 contraction)
F32 = mybir.dt.float32
BF16 = mybir.dt.bfloat16
GELU = mybir.ActivationFunctionType.Gelu
ADD = mybir.AluOpType.add
BF = ml_dtypes.bfloat16


def _chunks(total):
    """Token chunks: 512 wide (one fp32 PSUM bank); tail split in half so
    every chunk stays comfortably wide."""
    out, pos = [], 0
    while pos < total:
        rem = total - pos
        if rem > 512:
            w = 512 if rem >= 1024 or rem == 512 else rem // 2
        else:
            w = rem
        out.append((pos, w))
        pos += w
    return out


def _body(tc, x_d, w1_d, b1_d, w2_d, b2_d, o_d, t_tokens):
    nc = tc.nc
    with (
        tc.tile_pool(name="const", bufs=1) as constp,
        tc.tile_pool(name="wpool", bufs=1) as wp,
        tc.tile_pool(name="xt", bufs=3) as xtp,
        tc.tile_pool(name="ht", bufs=2) as htp,
        tc.tile_pool(name="onat", bufs=2) as onatp,
        tc.tile_pool(name="ps1", bufs=4, space=bass.MemorySpace.PSUM) as ps1p,
        tc.tile_pool(name="ps2", bufs=3, space=bass.MemorySpace.PSUM) as ps2p,
    ):
        b1_s = constp.tile([P, KF], F32)
        nc.sync.dma_start(out=b1_s[:], in_=b1_d)
        b2_s = constp.tile([P, KD], F32)
        nc.sync.dma_start(out=b2_s[:], in_=b2_d)

        w1_s = [wp.tile([P, FF], BF16, tag=f"w1_{k}", name=f"w1_{k}")
                for k in range(KD)]
        # W1 arrives in three 1024-col slices; the first unblocks fc1
        # m=0..7 while the rest stream in behind it. Finer slicing was
        # measured slower (DMA can't stay ahead of chain consumption).
        W1SLICES = [(0, 768), (768, 1536), (1536, 2304), (2304, 3072)]
        lo, hi = W1SLICES[0]
        for k in range(KD):
            nc.sync.dma_start(out=w1_s[k][:, lo:hi],
                              in_=w1_d[k * P:(k + 1) * P, lo:hi])

        # --- HAM warm-up: keep PE busy during the initial weight DMAs so
        # the clock gate is at 8/8 when real work starts. Reads an
        # uninitialized scratch tile (no DMA dependency; result discarded).
        scratch = constp.tile([P, 512], BF16)
        nc.vector.memset(scratch[:, :], 0.0)
        wps = ps1p.tile([P, 512], F32, tag="ps1", name="warm")
        for r in range(16):
            nc.tensor.matmul(
                wps[:, :], scratch[:, 0:P], scratch[:, :],
                start=True, stop=True, skip_group_check=True,
            )

        chunks = _chunks(t_tokens)

        def load_x(c0, cw):
            xts = []
            for k in range(KD):
                xn = xtp.tile([P, cw], BF16, tag=f"xt{k}", name=f"xt{k}")
                nc.sync.dma_start(
                    out=xn[:, :], in_=x_d[k * P:(k + 1) * P, c0:c0 + cw]
                )
                xts.append(xn)
            return xts

        def fc1(xts, cw):
            hts = []
            for m in range(KF):
                ps1 = ps1p.tile([P, cw], F32, tag="ps1")
                for k in range(KD):
                    nc.tensor.matmul(
                        ps1[:, :],
                        w1_s[k][:, m * P:(m + 1) * P],
                        xts[k][:, :],
                        start=(k == 0), stop=(k == KD - 1),
                    )
                ht = htp.tile([P, cw], BF16, tag=f"ht{m}")
                nc.scalar.activation(
                    ht[:, :], ps1[:, :], GELU, bias=b1_s[:, m:m + 1]
                )
                hts.append(ht)
            return hts

        def fc2(hts, c0, cw):
            for n in range(KD):
                ps2 = ps2p.tile([P, cw], F32, tag="ps2")
                for k in range(KF):
                    nc.tensor.matmul(
                        ps2[:, :],
                        w2_s[k][:, n * P:(n + 1) * P],
                        hts[k][:, :],
                        start=(k == 0), stop=(k == KF - 1),
                    )
                on = onatp.tile([P, cw], F32, tag="on")
                nc.vector.tensor_scalar(
                    out=on[:, :], in0=ps2[:, :],
                    scalar1=b2_s[:, n:n + 1], scalar2=None, op0=ADD,
                )
                nc.sync.dma_start(
                    out=o_d[n * P:(n + 1) * P, c0:c0 + cw], in_=on[:, :]
                )

        # Software-pipelined: emit fc1 of chunk c+1 before fc2 of chunk c so
        # the PE never waits on the gelu/ACT of the current chunk. DMA
        # emission order front-loads what the first chains need: W1 slice 0,
        # x chunk 0, rest of W1, then W2 (not needed until the first fc2).
        xts = load_x(*chunks[0])
        for lo, hi in W1SLICES[1:]:
            for k in range(KD):
                nc.sync.dma_start(
                    out=w1_s[k][:, lo:hi],
                    in_=w1_d[k * P:(k + 1) * P, lo:hi])
        hts = fc1(xts, chunks[0][1])
        w2_s = []
        for k in range(KF):
            w = wp.tile([P, DIM], BF16, tag=f"w2_{k}")
            nc.sync.dma_start(out=w[:], in_=w2_d[k * P:(k + 1) * P, :])
            w2_s.append(w)
        for ci, (c0, cw) in enumerate(chunks):
            if ci + 1 < len(chunks):
                nc0, ncw = chunks[ci + 1]
                nxts = load_x(nc0, ncw)
                nhts = fc1(nxts, ncw)
            fc2(hts, c0, cw)
            if ci + 1 < len(chunks):
                hts = nhts


def build_program(t_tokens=T):
    nc = bacc.Bacc("TRN2", target_bir_lowering=False, debug=False,
                   num_devices=NCORES)
    x_d = nc.dram_tensor("xt", [DIM, t_tokens], BF16, kind="ExternalInput").ap()
    w1_d = nc.dram_tensor("w1t", [DIM, FF], BF16, kind="ExternalInput").ap()
    b1_d = nc.dram_tensor("b1", [P, KF], F32, kind="ExternalInput").ap()
    w2_d = nc.dram_tensor("w2t", [FF, DIM], BF16, kind="ExternalInput").ap()
    b2_d = nc.dram_tensor("b2", [P, KD], F32, kind="ExternalInput").ap()
    o_d = nc.dram_tensor("out", [DIM, t_tokens], F32,
                         kind="ExternalOutput").ap()
    with tile.TileContext(nc) as tc:
        _body(tc, x_d, w1_d, b1_d, w2_d, b2_d, o_d, t_tokens)
    nc.compile()
    return nc


def host_prep(x, W1, b1, W2, b2, mask1, mask2):
    xT = np.ascontiguousarray(
        np.asarray(x, dtype=np.float32).reshape(TOK, DIM).T).astype(BF)
    m1 = np.repeat(np.repeat(np.asarray(mask1, dtype=bool), BLK, 0), BLK, 1)
    m2 = np.repeat(np.repeat(np.asarray(mask2, dtype=bool), BLK, 0), BLK, 1)
    w1t = np.ascontiguousarray(
        (np.asarray(W1, np.float32) * m1.astype(np.float32)).T
    ).astype(BF)                                                  # [DIM, FF]
    w2t = np.ascontiguousarray(
        (np.asarray(W2, np.float32) * m2.astype(np.float32)).T
    ).astype(BF)                                                  # [FF, DIM]
    b1h = np.ascontiguousarray(
        np.asarray(b1, np.float32).reshape(KF, P).T)              # [P, KF]
    b2h = np.ascontiguousarray(
        np.asarray(b2, np.float32).reshape(KD, P).T)              # [P, KD]
    return xT, w1t, b1h, w2t, b2h


_PROGRAM = None


def _get_program():
    global _PROGRAM
    if _PROGRAM is None:
        _PROGRAM = build_program(T)
    return _PROGRAM


def kernel(x, W1, b1, W2, b2, mask1, mask2, **run_kwargs):
    xT, w1t, b1h, w2t, b2h = host_prep(x, W1, b1, W2, b2, mask1, mask2)
    nc = _get_program()
    in_maps = [
        {"xt": np.ascontiguousarray(xT[:, c * T:(c + 1) * T]), "w1t": w1t,
         "b1": b1h, "w2t": w2t, "b2": b2h}
        for c in range(NCORES)
    ]
    res = run_bass_kernel_spmd(nc, in_maps, list(range(NCORES)), **run_kwargs)
    outT = np.concatenate(
        [res.results[c]["out"] for c in range(NCORES)], axis=1)   # [DIM, TOK]
    out = np.ascontiguousarray(outT.T).reshape(B, S, DIM).astype(np.float32)
    if run_kwargs:
        kernel.last_results = res
    return out
